# revision 1
# baseline (speedup 1.0000x reference)
"""Trainium2 Bass kernel for nn_DecoderBlock_Mamba (AxialDW conv + 1x1 conv +
BN + ReLU + LN + Mamba selective scan + residual).

Sharding: 8 cores = (batch b in 0..3) x (state-half sigma in {0,1}).
Each core runs the full per-image pipeline for its batch element but only 8 of
the 16 SSM states; partial y is AllReduce'd within core pairs, post-stack is
computed redundantly on both cores of a pair.

Self-contained: hardcodes all shapes; no sibling imports.
"""
import numpy as np

C = 64
DI = 128
DS = 16
DR = 4
B = 4
H = 64
W = 64
L = H * W
NS = 8            # states per core
NCORES = 8
ROW = W + 2       # padded row stride
LP = (H + 2) * ROW
NCH = 8           # L chunks of 512
CH = 512
EPS = 1e-5

_cached = {}


def _build_program(sim=False, phases=3):
    import concourse.bass as bass
    import concourse.bacc as bacc
    import concourse.mybir as mybir
    import concourse.tile as tile

    dt = mybir.dt
    f32 = dt.float32
    bf16 = dt.bfloat16
    Act = mybir.ActivationFunctionType
    Alu = mybir.AluOpType
    Axis = mybir.AxisListType

    nc = bacc.Bacc(None, target_bir_lowering=False)

    def din(name, shape, dtype=f32):
        return nc.dram_tensor(name, shape, dtype, kind="ExternalInput")

    ximgs_d = din("ximgs", [C, 5 * L], bf16)
    cf32_d = din("cf32", [128, 19])
    cbf_d = din("cbf", [128, 2948], bf16)

    out_d = nc.dram_tensor("out_f", [C, L], f32, kind="ExternalOutput")

    groups = [[0, 1], [2, 3], [4, 5], [6, 7]]

    with tile.TileContext(nc) as tc:
        with (
            tc.tile_pool(name="dram", bufs=1, space="DRAM") as dpool,
            tc.tile_pool(name="const", bufs=1) as cpool,
            tc.tile_pool(name="big", bufs=1) as bpool,
            tc.tile_pool(name="sm", bufs=2) as spool,
            tc.tile_pool(name="da", bufs=2) as dapool,
            tc.tile_pool(name="dbx", bufs=2) as dbxpool,
            tc.tile_pool(name="ps", bufs=4, space="PSUM") as ps,
            tc.tile_pool(name="psy", bufs=2, space="PSUM") as psy,
        ):
            # ---- load constants (packed: 3 DMAs total) ----
            cf = cpool.tile([128, 19], f32)
            cb = cpool.tile([128, 2948], bf16)
            nc.sync.dma_start(cf[:], cf32_d[:])
            nc.sync.dma_start(cb[:], cbf_d[:])
            bn_s = cf[0:C, 0:1]
            bn_b = cf[0:C, 1:2]
            ip_b = cf[:, 2:4]
            cd_w = cf[:, 4:8]
            cd_b = cf[:, 8:9]
            dt_b = cf[:, 9:10]
            a_sc = cf[:, 10:18]
            Dp = cf[:, 18:19]
            ident = cb[:, 0:128]
            cw = cb[0:C, 128:448]
            ip_lhsT = cb[0:C, 448:704]
            xpdt_lhsT = cb[:, 704:708]
            dt_lhsT = cb[0:DR, 708:836]
            brep_lhsT = cb[:, 836:1860]
            crep_lhsT = cb[:, 1860:2884]
            op_lhsT = cb[:, 2884:2948]

            # ---- persistent activations ----
            SEQ = bpool.tile([C, L], bf16)           # BN+ReLU output (residual)
            HN = bpool.tile([C, L], bf16)            # LN-normalized (no affine)
            XM0 = bpool.tile([DI, L + 4], bf16)      # conv1d input, data @ col 4
            ZS = bpool.tile([DI, L], bf16)           # silu(z)
            XC = bpool.tile([DI, L], bf16)
            DT = bpool.tile([DI, L], bf16)
            U = bpool.tile([DI, L], bf16)
            Hs = [bpool.tile([DI, L], bf16, name=f"H{j}", tag=f"H{j}") for j in range(NS)]
            YSUM = bpool.tile([DI, L], bf16, name="YSUM", tag="U")

            # Prime ACT's vector clock on the const DMAs so later
            # activations (limited wait slots) don't re-wait on them.
            warm = cpool.tile([128, 1], f32, tag="warm")
            nc.scalar.activation(warm[:], cf[:, 0:1], Act.Copy)
            warm2 = cpool.tile([128, 1], bf16, tag="warm2")
            nc.scalar.activation(warm2[:], cb[:, 0:1], Act.Copy)
            eps_t = cpool.tile([128, 1], f32, tag="epsl")
            nc.gpsimd.memset(eps_t[:], EPS)
            nc.vector.tensor_scalar_mul(XM0[:, 0:4], cf[:, 0:4], 0.0)

            IMGS = [bpool.tile([C, L], bf16, name=f"img{t}", tag=f"H{t}")
                    for t in range(5)]
            for t in range(5):
                nc.sync.dma_start(IMGS[t][:], ximgs_d[:, t * L:(t + 1) * L])

            # ---- front conv: 5 accumulating taps + BN + ReLU ----
            for chi in range(NCH):
                sl = slice(chi * CH, (chi + 1) * CH)
                pc = ps.tile([C, CH], f32, tag="mm")
                for tap in range(5):
                    nc.tensor.matmul(pc[:], cw[:, tap * C:(tap + 1) * C],
                                     IMGS[tap][:, sl],
                                     start=(tap == 0), stop=(tap == 4))
                nc.scalar.activation(SEQ[:, chi * CH:(chi + 1) * CH], pc[:],
                                     Act.Relu, bias=bn_b, scale=bn_s)

            # ---- LayerNorm over channels, batched 4 blocks per DVE op ----
            HN0 = bpool.tile([128, L // 2], bf16, name="HN0", tag="HN0")
            VARS = spool.tile([128, 32], f32, tag="VARS")
            NG = L // 512  # 8 groups of 4 128-token blocks
            for g in range(NG if phases >= 1 else 0):
                tps4 = ps.tile([128, 4, C], bf16, tag="mm")
                for k in range(4):
                    blk = g * 4 + k
                    nc.tensor.transpose(tps4[:, k, :],
                                        SEQ[:, blk * 128:(blk + 1) * 128],
                                        ident[0:C, 0:C])
                mu4 = spool.tile([128, 4], f32, tag="mu4")
                nc.vector.tensor_reduce(mu4[:], tps4[:], Axis.X, Alu.add)
                mun4 = spool.tile([128, 4], f32, tag="mun4")
                nc.vector.tensor_scalar_mul(mun4[:], mu4[:], 1.0 / C)
                h04 = HN0[:, g * 256:(g + 1) * 256].rearrange(
                    "p (b c) -> p b c", b=4)
                nc.vector.tensor_tensor(h04, tps4[:],
                                        mun4[:].to_broadcast((128, 4, C)),
                                        op=Alu.subtract)
                sq4 = spool.tile([128, 4, C], f32, tag="sq4")
                nc.vector.tensor_mul(sq4[:], h04, h04)
                ssq4 = spool.tile([128, 4], f32, tag="ssq4")
                nc.vector.tensor_reduce(ssq4[:], sq4[:], Axis.X, Alu.add)
                nc.vector.tensor_scalar(VARS[:, g * 4:(g + 1) * 4], ssq4[:],
                                        1.0 / C, EPS,
                                        op0=Alu.mult, op1=Alu.add)
            SQV = spool.tile([128, 32], f32, tag="SQV")
            RSTD = spool.tile([128, 32], f32, tag="RSTD")
            if phases >= 1:
                nc.scalar.activation(SQV[:], VARS[:], Act.Sqrt)
                nc.vector.reciprocal(RSTD[:], SQV[:])
            HNT = bpool.tile([128, L // 2], bf16, name="HNT", tag="HNT")
            for g in range(NG if phases >= 1 else 0):
                hnT4 = HNT[:, g * 256:(g + 1) * 256].rearrange(
                    "p (b c) -> p b c", b=4)
                nc.vector.tensor_tensor(
                    hnT4, HN0[:, g * 256:(g + 1) * 256].rearrange(
                        "p (b c) -> p b c", b=4),
                    RSTD[:, g * 4:(g + 1) * 4].to_broadcast((128, 4, C)),
                    op=Alu.mult)
                tb4 = ps.tile([C, 4, 128], bf16, tag="mm")
                for k in range(4):
                    blk = g * 4 + k
                    nc.tensor.transpose(tb4[:, k, :],
                                        HNT[:, blk * C:(blk + 1) * C],
                                        ident)
                nc.scalar.activation(HN[:, g * CH:(g + 1) * CH],
                                     tb4[:].rearrange("p a b -> p (a b)"),
                                     Act.Copy)
            # ---- in_proj ----
            for chi in range(NCH if phases >= 1.5 else 0):
                sl = slice(chi * CH, (chi + 1) * CH)
                xm_ps = ps.tile([DI, CH], f32, tag="mm")
                z_ps = ps.tile([DI, CH], f32, tag="mm")
                nc.tensor.matmul(xm_ps[:], ip_lhsT[0:C, 0:DI], HN[:, sl],
                                 start=True, stop=True)
                nc.tensor.matmul(z_ps[:], ip_lhsT[0:C, DI:2 * DI], HN[:, sl],
                                 start=True, stop=True)
                nc.scalar.activation(XM0[:, 4 + chi * CH:4 + (chi + 1) * CH],
                                     xm_ps[:], Act.Identity, bias=ip_b[:, 0:1])
                nc.scalar.activation(ZS[:, sl], z_ps[:], Act.Silu,
                                     bias=ip_b[:, 1:2])
            # ---- causal conv1d (4 taps) + silu ----
            # xc_t = sum_k w_k * xm_{t-3+k}; XM0 holds xm at col 4,
            # XM1 at col 3: tap k reads XM0[1+k:] or XM1[k:] — use whichever
            # start offset is even so bf16 ops keep 4B alignment.
            ACC1 = bpool.tile([DI, L], bf16, name="ACC1", tag="ACC1")
            ACC2 = bpool.tile([DI, L], bf16, name="ACC2", tag="ACC2")
            if phases < 2:
                nc.gpsimd.dma_start(out_d[:, 0:CH], SEQ[:, 0:CH])
            if phases >= 2:
                nc.vector.tensor_scalar_mul(ACC1[:], XM0[:, 1:1 + L], cd_w[:, 0:1])
                nc.vector.scalar_tensor_tensor(ACC2[:], XM0[:, 2:2 + L], cd_w[:, 1:2],
                                               ACC1[:], op0=Alu.mult, op1=Alu.add)
                nc.vector.scalar_tensor_tensor(ACC1[:], XM0[:, 3:3 + L], cd_w[:, 2:3],
                                               ACC2[:], op0=Alu.mult, op1=Alu.add)
                nc.vector.scalar_tensor_tensor(ACC2[:], XM0[:, 4:4 + L], cd_w[:, 3:4],
                                               ACC1[:], op0=Alu.mult, op1=Alu.add)
                nc.scalar.activation(XC[:], ACC2[:], Act.Silu, bias=cd_b)

            # ---- x_proj (dt rows) + dt_proj + softplus ----
            for chi in range(NCH if phases >= 2 else 0):
                sl = slice(chi * CH, (chi + 1) * CH)
                dtr_ps = ps.tile([DR, CH], f32, tag="mm")
                nc.tensor.matmul(dtr_ps[:], xpdt_lhsT, XC[:, sl],
                                 start=True, stop=True)
                dtr_sb = spool.tile([DR, CH], bf16, tag="dtrsb")
                nc.scalar.activation(dtr_sb[:], dtr_ps[:], Act.Copy)
                dt_ps = ps.tile([DI, CH], f32, tag="mm")
                nc.tensor.matmul(dt_ps[:], dt_lhsT, dtr_sb[:],
                                 start=True, stop=True)
                esb = spool.tile([DI, CH], f32, tag="esb")
                nc.scalar.activation(esb[:], dt_ps[:], Act.Exp, bias=dt_b)
                nc.scalar.activation(DT[:, sl], esb[:], Act.Ln, bias=1.0)
            if phases >= 2:
                nc.vector.tensor_mul(U[:], DT[:], XC[:])

            # ---- per-state: dA = exp(a_j*dt), dBx = u*B_j, scan ----
            LH = L // 2
            for half in range(2):
                for j in range(NS if phases >= 2.5 else 0):
                    hsl = slice(half * LH, (half + 1) * LH)
                    dA = dapool.tile([DI, LH], f32, tag="dA")
                    nc.scalar.activation(dA[:], DT[:, hsl], Act.Exp,
                                         scale=a_sc[:, j:j + 1])
                    dbx = dbxpool.tile([DI, LH], bf16, tag="dbx")
                    for ci in range(LH // CH):
                        sl = slice(half * LH + ci * CH,
                                   half * LH + (ci + 1) * CH)
                        lsl = slice(ci * CH, (ci + 1) * CH)
                        br = ps.tile([DI, CH], f32, tag="mm")
                        nc.tensor.matmul(br[:], brep_lhsT[:, j * DI:(j + 1) * DI],
                                         XC[:, sl], start=True, stop=True)
                        nc.vector.tensor_tensor(dbx[:, lsl], U[:, sl], br[:],
                                                op=Alu.mult)
                    init = 0.0 if half == 0 else Hs[j][:, LH - 1:LH]
                    nc.vector.tensor_tensor_scan(Hs[j][:, hsl], dA[:], dbx[:],
                                                 init, op0=Alu.mult, op1=Alu.add)

            # ---- y accumulation: y = sum_j H_j * C_j  (PE-accumulated) ----
            y_in_t = dpool.tile([DI, L], bf16, tag="yin")
            y_out_t = dpool.tile([DI, L], bf16, tag="yout")
            for chi in range(NCH if phases >= 3 else 0):
                sl = slice(chi * CH, (chi + 1) * CH)
                yps = psy.tile([DI, CH], f32, tag="yps")
                for j in range(NS):
                    cr = ps.tile([DI, CH], f32, tag="mm")
                    nc.tensor.matmul(cr[:], crep_lhsT[:, j * DI:(j + 1) * DI],
                                     XC[:, sl], start=True, stop=True)
                    tmp = spool.tile([DI, CH], bf16, tag="ymul")
                    nc.vector.tensor_tensor(tmp[:], Hs[j][:, sl], cr[:],
                                            op=Alu.mult)
                    nc.tensor.matmul(yps[:], ident, tmp[:],
                                     start=(j == 0), stop=(j == NS - 1))
                ysb = spool.tile([DI, CH], bf16, tag="ysb")
                nc.scalar.activation(ysb[:], yps[:], Act.Copy)
                nc.sync.dma_start(y_in_t[:, sl], ysb[:])

            # ---- AllReduce partial y within batch pair (2 halves) ----
            if sim or phases < 3:
                nc.sync.dma_start(y_out_t[:], y_in_t[:])
            else:
                nc.gpsimd.collective_compute(
                    "AllReduce", Alu.add, replica_groups=groups,
                    ins=[y_in_t.opt()], outs=[y_out_t.opt()])
            nc.sync.dma_start(YSUM[:], y_out_t[:])

            # ---- post: ys = (y + xc*Dp) * silu(z); out = op(ys) + seq ----
            XCD = bpool.tile([DI, L], bf16, name="XCD", tag="DT")
            YS = bpool.tile([DI, L], bf16, tag="YS")
            for hf in range(2):
                hsl2 = slice(hf * (L // 2), (hf + 1) * (L // 2))
                nc.vector.tensor_scalar_mul(XCD[:, hsl2], XC[:, hsl2], Dp)
                nc.vector.tensor_add(XCD[:, hsl2], YSUM[:, hsl2], XCD[:, hsl2])
                nc.vector.tensor_mul(YS[:, hsl2], XCD[:, hsl2], ZS[:, hsl2])
            OUT = bpool.tile([C, L], f32, name="OUT", tag="XM0")
            for chi in range(NCH):
                sl = slice(chi * CH, (chi + 1) * CH)
                op_ps = ps.tile([C, CH], f32, tag="mm")
                nc.tensor.matmul(op_ps[:], op_lhsT, YS[:, sl],
                                 start=True, stop=True)
                nc.vector.tensor_tensor(OUT[:, sl], op_ps[:], SEQ[:, sl],
                                        op=Alu.add)
                nc.sync.dma_start(out_d[:, sl], OUT[:, sl])

    nc.compile()
    return nc


def _host_precompute(inp):
    import ml_dtypes
    f = lambda k: np.asarray(inp[k], np.float32)
    bf = lambda a: np.ascontiguousarray(a.astype(ml_dtypes.bfloat16))
    w1 = f("conv_w")[:, :, 0, 0]
    wh = f("dwh_w")[:, 0, :, 0]
    ww = f("dww_w")[:, 0, 0, :]
    taps = [
        w1 * (1.0 + wh[:, 1] + ww[:, 1])[None, :],   # center
        w1 * wh[:, 0][None, :],                       # up
        w1 * wh[:, 2][None, :],                       # down
        w1 * ww[:, 0][None, :],                       # left
        w1 * ww[:, 2][None, :],                       # right
    ]
    cw = np.concatenate([t.T for t in taps], axis=1)  # [cin=64, 5*64]
    btot = f("conv_b") + w1 @ (f("dwh_b") + f("dww_b"))
    s_bn = f("bn_g") / np.sqrt(f("bn_v") + EPS)
    bn_bias = s_bn * (btot - f("bn_m")) + f("bn_b")
    ipw = f("in_proj_w")
    ip_lhsT = (ipw * f("ln_g")[None, :]).T            # [64, 256]
    ip_bias = ipw @ f("ln_b")                          # [256]
    xpw = f("x_proj_w")                                # [36, 128]
    a_full = -np.exp(np.asarray(inp["A_log"], np.float32))  # [DI, DS]

    per_sigma = []
    for sg in range(2):
        s_lo = sg * NS
        cf32 = np.zeros((128, 19), np.float32)
        cf32[:C, 0] = s_bn
        cf32[:C, 1] = bn_bias
        cf32[:, 2] = ip_bias[:DI]
        cf32[:, 3] = ip_bias[DI:]
        cf32[:, 4:8] = f("convd_w")[:, 0, :]
        cf32[:, 8] = f("convd_b")
        cf32[:, 9] = f("dt_proj_b")
        for j in range(NS):
            cf32[:, 10 + j] = a_full[:, s_lo + j]
        cf32[:, 18] = f("Dp")

        cbf = np.zeros((128, 2948), np.float32)
        cbf[:, 0:128] = np.eye(128, dtype=np.float32)
        cbf[:C, 128:448] = cw
        cbf[:C, 448:704] = ip_lhsT
        cbf[:, 704:708] = xpw[:DR].T
        cbf[:DR, 708:836] = f("dt_proj_w").T
        for j in range(NS):
            s = s_lo + j
            cbf[:, 836 + j * DI:836 + (j + 1) * DI] = xpw[DR + s][:, None]
            cbf[:, 1860 + j * DI:1860 + (j + 1) * DI] = xpw[DR + DS + s][:, None]
        cbf[:, 2884:2948] = f("out_proj_w").T
        per_sigma.append(dict(cf32=cf32, cbf=bf(cbf)))
    return {}, per_sigma


def _shift_images(xb):
    # 5 pre-shifted copies: ctr, up(reads h-1), dn(h+1), lf(w-1), rt(w+1)
    import ml_dtypes
    out = np.zeros((C, 5, H, W), np.float32)
    out[:, 0] = xb
    out[:, 1, 1:, :] = xb[:, :-1, :]
    out[:, 2, :-1, :] = xb[:, 1:, :]
    out[:, 3, :, 1:] = xb[:, :, :-1]
    out[:, 4, :, :-1] = xb[:, :, 1:]
    return np.ascontiguousarray(
        out.transpose(1, 0, 2, 3).reshape(5, C, L).transpose(1, 0, 2)
        .reshape(C, 5 * L).astype(ml_dtypes.bfloat16))


TRACE = False
LAST_EXEC_NS = None
LAST_TRACE_DIR = None


def kernel(**inputs):
    global LAST_EXEC_NS, LAST_TRACE_DIR
    from concourse.bass_utils import run_bass_kernel_spmd

    if "nc" not in _cached:
        _cached["nc"] = _build_program()
    nc = _cached["nc"]

    common, per_sigma = _host_precompute(inputs)
    x = np.asarray(inputs["x"], np.float32)
    in_maps = []
    for c in range(NCORES):
        b, sg = c // 2, c % 2
        m = dict(common)
        m.update(per_sigma[sg])
        m["ximgs"] = _shift_images(x[b])
        in_maps.append(m)

    kw = {}
    if TRACE:
        import tempfile
        LAST_TRACE_DIR = tempfile.mkdtemp(prefix="bass_trace_")
        kw = dict(trace=True, tmpdir=LAST_TRACE_DIR)
    r = run_bass_kernel_spmd(nc, in_maps, list(range(NCORES)), **kw)
    if r.exec_time_ns is not None:
        LAST_EXEC_NS = r.exec_time_ns
    res = r.results
    out = np.empty((B, C, H, W), np.float32)
    for b in range(B):
        out[b] = np.asarray(res[2 * b]["out_f"], np.float32).reshape(C, H, W)
    return out



# revision 2
# speedup vs baseline: 1.5517x; 1.5517x over previous
"""Trainium2 Bass kernel v4 for nn_DecoderBlock_Mamba.

Sharding: 8 cores = (batch b in 0..3) x (state-half sigma in {0,1}).
Each core runs the full per-image pipeline for its batch element but only 8 of
the 16 SSM states; partial y is AllReduce'd within core pairs.

Structure:
- causal conv1d folded into in_proj (4 shifted accumulated matmuls with
  host-precomputed diag(w_k) @ W weights; K=3 boundary-correction matmul)
- dt_proj @ x_proj_dt folded into one host matrix (rank-4 [128,128])
- b/c broadcasts via transpose-mode matmuls -> bf16 PSUM, dbx/y mults at
  DVE 2x; 3 states' dbx+scan and 2 states' y mult run on GPSIMD
- front conv + LayerNorm chunk-interleaved; per-group rstd via sqrt table
  (relu/square/sqrt all live in one act table); silus emitted after all
  sqrt-table ops so only ~5 act-table loads happen
- back half (dA/dbx/scan, y, AllReduce, post, out_proj) runs in 2
  L/2-segments, software-pipelined: y of segment 0 overlaps scans of
  segment 1, AllReduce latency hides under compute
- LN ssq reduces + HNP copies on GPSIMD during phase 1

Self-contained: hardcodes all shapes; no sibling imports.
"""
import numpy as np

C = 64
DI = 128
DS = 16
DR = 4
B = 4
H = 64
W = 64
L = H * W
NS = 8            # states per core
NCORES = 8
NCH = 8           # L chunks of 512
CH = 512
SEG = 2048
EPS = 1e-5
NPOOL = 4         # states whose dbx mult runs on gpsimd
NPOOLY = 3        # states whose y mult runs on gpsimd
NCB = 3472        # cbf columns

_cached = {}


def _build_program(sim=False):
    import concourse.bass as bass
    import concourse.bacc as bacc
    import concourse.mybir as mybir
    import concourse.tile as tile

    dt = mybir.dt
    f32 = dt.float32
    bf16 = dt.bfloat16
    Act = mybir.ActivationFunctionType
    Alu = mybir.AluOpType
    Axis = mybir.AxisListType

    nc = bacc.Bacc(None, target_bir_lowering=False)

    def din(name, shape, dtype=f32):
        return nc.dram_tensor(name, shape, dtype, kind="ExternalInput")

    ximgs_d = din("ximgs", [C, 5 * L], bf16)
    cf32_d = din("cf32", [128, 16])
    cbf_d = din("cbf", [128, NCB], bf16)

    out_d = nc.dram_tensor("out_f", [C, L], f32, kind="ExternalOutput")

    groups = [[0, 1], [2, 3], [4, 5], [6, 7]]
    # emission order of states: pool states first so their long chain starts
    # early
    order = [4, 0, 5, 1, 6, 2, 7, 3]

    with tile.TileContext(nc) as tc:
        with (
            tc.tile_pool(name="dram", bufs=1, space="DRAM") as dpool,
            tc.tile_pool(name="const", bufs=1) as cpool,
            tc.tile_pool(name="big", bufs=1) as bpool,
            tc.tile_pool(name="sm", bufs=2) as spool,
            tc.tile_pool(name="da", bufs=2) as dapool,
            tc.tile_pool(name="dbx", bufs=2) as dbxpool,
            tc.tile_pool(name="ymm", bufs=2) as ypool,
            tc.tile_pool(name="brc", bufs=1) as brcpool,
            tc.tile_pool(name="ps", bufs=2, space="PSUM") as ps,
            tc.tile_pool(name="bc", bufs=4, space="PSUM") as pbc,
            tc.tile_pool(name="psy", bufs=2, space="PSUM") as psy,
        ):
            # ---- constants (front-needed cols first) ----
            cf = cpool.tile([128, 16], f32)
            cb = cpool.tile([128, NCB], bf16)
            nc.sync.dma_start(cf[:], cf32_d[:])
            nc.sync.dma_start(cb[:, 0:448], cbf_d[:, 0:448])
            nc.sync.dma_start(cb[:, 448:NCB], cbf_d[:, 448:NCB])
            bn_s = cf[0:C, 0:1]
            bn_b = cf[0:C, 1:2]
            b_z = cf[:, 2:3]
            b_xc = cf[:, 3:4]
            dt_b = cf[:, 4:5]
            a_sc = cf[:, 5:13]
            Dp = cf[:, 13:14]
            ident = cb[:, 0:128]
            cw = cb[0:C, 128:448]
            ipz_lhsT = cb[0:C, 448:576]
            wk_lhsT = cb[0:C, 576:1088]       # 4 x [64,128]
            dtM_lhsT = cb[:, 1088:1216]
            brep_lhsT = cb[:, 1216:2240]      # 8 x [128,128]
            crep_lhsT = cb[:, 2240:3264]
            op_lhsT = cb[:, 3264:3328]
            corrT = cb[0:3, 3328:3456]        # [3,128]
            bc_lhsT = cb[:, 3456:3472]        # [128,16] B rows 0-7, C rows 8-15

            # ---- persistent activations ----
            SEQ = bpool.tile([C, L], bf16)
            HNP = bpool.tile([C, L + 3], bf16)     # ln-normed, 3 zero pad cols
            hnT = bpool.tile([128, 2048], bf16, name="hnT", tag="BCROWS")
            XC = bpool.tile([DI, L], bf16)
            DT = bpool.tile([DI, L], bf16)
            U = bpool.tile([DI, L], bf16)
            ZS = bpool.tile([DI, L], bf16)
            Hs = [bpool.tile([DI, L], bf16, name=f"H{j}", tag=f"H{j}")
                  for j in range(NS)]
            YSUM = bpool.tile([DI, L], bf16)
            SQ32 = bpool.tile([128, 2048], f32, name="SQ32", tag="SCR")
            esb = bpool.tile([DI, L], bf16, name="esb", tag="SCR")
            XCD = bpool.tile([DI, L], bf16, name="XCD", tag="SCR")
            YS = bpool.tile([DI, L], bf16, name="YS", tag="YS")
            ysb = bpool.tile([DI, L], bf16, name="ysb", tag="YS")
            BCROWS = bpool.tile([16, L], bf16, name="BCROWS", tag="BCROWS")
            MU = spool.tile([128, 32], f32, tag="MU")
            SUM32 = spool.tile([128, 32], f32, tag="SUM32")
            SSQ32 = spool.tile([128, 32], f32, tag="SSQ32")
            VAR = spool.tile([128, 32], f32, tag="VAR")
            SQV = spool.tile([128, 32], f32, tag="SQV")
            RSTD = spool.tile([128, 32], f32, tag="RSTD")

            IMGS = [bpool.tile([C, L], bf16, name=f"img{t}", tag=f"H{t}")
                    for t in range(5)]
            for h in range(4):
                for t in range(5):
                    nc.sync.dma_start(
                        IMGS[t][:, h * 1024:(h + 1) * 1024],
                        ximgs_d[:, t * L + h * 1024:t * L + (h + 1) * 1024])

            nc.gpsimd.memset(HNP[:, 0:3], 0.0)

            # Prime ACT's vector clock on the const DMAs (limited wait slots).
            warm = cpool.tile([128, 1], f32, tag="warm")
            nc.scalar.activation(warm[:], cf[:, 0:1], Act.Copy)
            warm2 = cpool.tile([128, 1], bf16, tag="warm2")
            nc.scalar.activation(warm2[:], cb[:, 0:1], Act.Copy)

            # ---- phase 1a: front conv + LayerNorm, chunk-interleaved ----
            # ACT functions here: Relu, Square, Sqrt, Copy (one table)
            for g in range(NCH):
                sl = slice(g * CH, (g + 1) * CH)
                pc = ps.tile([C, CH], f32, tag="mm")
                for tap in range(5):
                    nc.tensor.matmul(pc[:], cw[:, tap * C:(tap + 1) * C],
                                     IMGS[tap][:, sl],
                                     start=(tap == 0), stop=(tap == 4))
                nc.scalar.activation(SEQ[:, sl], pc[:],
                                     Act.Relu, bias=bn_b, scale=bn_s)
                g4 = slice(g * 4, (g + 1) * 4)
                tps4 = psy.tile([128, 4, C], bf16, tag="y")
                for k in range(4):
                    blk = g * 4 + k
                    nc.tensor.transpose(tps4[:, k, :],
                                        SEQ[:, blk * 128:(blk + 1) * 128],
                                        ident[0:C, 0:C])
                nc.vector.tensor_reduce(SUM32[:, g4], tps4[:], Axis.X, Alu.add)
                nc.scalar.activation(SQ32[:, g * 256:(g + 1) * 256], tps4[:],
                                     Act.Square)
                nc.vector.tensor_reduce(
                    SSQ32[:, g4],
                    SQ32[:, g * 256:(g + 1) * 256].rearrange(
                        "p (b c) -> p b c", b=4),
                    Axis.X, Alu.add)
                nc.vector.tensor_scalar_mul(MU[:, g4], SUM32[:, g4], 1.0 / C)
                MUSQ = spool.tile([128, 4], f32, tag="MUSQ")
                nc.vector.tensor_mul(MUSQ[:], MU[:, g4], MU[:, g4])
                nc.vector.tensor_scalar(VAR[:, g4], SSQ32[:, g4], 1.0 / C, EPS,
                                        op0=Alu.mult, op1=Alu.add)
                nc.vector.tensor_tensor(VAR[:, g4], VAR[:, g4], MUSQ[:],
                                        op=Alu.subtract)
                nc.scalar.activation(SQV[:, g4], VAR[:, g4], Act.Sqrt)
                nc.vector.reciprocal(RSTD[:, g4], SQV[:, g4])
                for k in range(4):
                    blk = g * 4 + k
                    nc.vector.tensor_scalar(
                        hnT[:, blk * C:(blk + 1) * C], tps4[:, k, :],
                        MU[:, blk:blk + 1], RSTD[:, blk:blk + 1],
                        op0=Alu.subtract, op1=Alu.mult)
                tb4 = pbc.tile([C, 4, 128], bf16, tag="bc")
                for k in range(4):
                    blk = g * 4 + k
                    nc.tensor.transpose(tb4[:, k, :],
                                        hnT[:, blk * C:(blk + 1) * C],
                                        ident)
                nc.vector.tensor_copy(HNP[:, 3 + g * CH:3 + (g + 1) * CH],
                                      tb4[:].rearrange("p a b -> p (a b)"))

            # ---- phase 1b: in_proj + folded conv1d + z (silu table) ----
            for chi in range(NCH):
                sl = slice(chi * CH, (chi + 1) * CH)
                xc_ps = ps.tile([DI, CH], f32, tag="mm")
                for k in range(4):
                    nc.tensor.matmul(xc_ps[:], wk_lhsT[:, k * 128:(k + 1) * 128],
                                     HNP[:, k + chi * CH:k + chi * CH + CH],
                                     start=(k == 0),
                                     stop=(k == 3 and chi != 0))
                if chi == 0:
                    nc.tensor.matmul(xc_ps[:, 0:3], corrT,
                                     ident[0:3, 0:3],
                                     start=False, stop=True)
                nc.scalar.activation(XC[:, sl], xc_ps[:], Act.Silu, bias=b_xc)
                z_ps = ps.tile([DI, CH], f32, tag="mm")
                nc.tensor.matmul(z_ps[:], ipz_lhsT,
                                 HNP[:, 3 + chi * CH:3 + (chi + 1) * CH],
                                 start=True, stop=True)
                nc.scalar.activation(ZS[:, sl], z_ps[:], Act.Silu, bias=b_z)

            # ---- phase 1c: dt path (exp chunks, then chunked ln + U) ----
            for chi in range(NCH):
                sl = slice(chi * CH, (chi + 1) * CH)
                dt_ps = ps.tile([DI, CH], f32, tag="mm")
                nc.tensor.matmul(dt_ps[:], dtM_lhsT, XC[:, sl],
                                 start=True, stop=True)
                nc.scalar.activation(esb[:, sl], dt_ps[:], Act.Exp, bias=dt_b)
                nc.vector.tensor_scalar_mul(XCD[:, sl], XC[:, sl], Dp)
                bc_ps = pbc.tile([16, CH], bf16, tag="bc")
                nc.tensor.matmul(bc_ps[:], bc_lhsT, XC[:, sl],
                                 is_transpose=True)
                nc.scalar.activation(BCROWS[:, sl], bc_ps[:], Act.Copy)
            for chi in range(NCH):
                sl = slice(chi * CH, (chi + 1) * CH)
                nc.scalar.activation(DT[:, sl], esb[:, sl], Act.Ln, bias=1.0)
                nc.vector.tensor_mul(U[:, sl], DT[:, sl], XC[:, sl])

            # ---- phase 2: per-segment scan + y + AR + post + out ----
            y_in_segs = [dpool.tile([DI, n * CH], bf16, name=f"y_in{i}", tag=f"yin{i}")
                         for i, n in enumerate((3, 3, 2))]
            y_out_segs = [dpool.tile([DI, n * CH], bf16, name=f"y_out{i}", tag=f"yout{i}")
                          for i, n in enumerate((3, 3, 2))]
            bc_d = dpool.tile([16, L], bf16, tag="bcd")
            OUTH = [bpool.tile([C, 1536], f32, name=f"outh{h}",
                                tag=f"OUTH{h % 2}") for h in range(3)]
            POOLJ = list(range(NS - NPOOL, NS))     # dbx+scan on gpsimd
            POOLYJ = list(range(NS - NPOOLY, NS))   # y mult on gpsimd
            DVEYJ = [j for j in range(NS) if j not in POOLYJ]

            def y_chunk(c, crps, c0):
                ysl = slice(c * CH, (c + 1) * CH)
                lsl = slice((c - c0) * CH, (c - c0 + 1) * CH)
                yps = psy.tile([DI, CH], f32, tag="y")
                # pool states first: their ymm comes from SBUF crp tiles and
                # can start immediately, overlapping the cr matmuls below
                for i, j in enumerate(POOLYJ):
                    ymm = ypool.tile([DI, CH], bf16, tag="ymmp")
                    nc.gpsimd.tensor_tensor(ymm[:], Hs[j][:, ysl],
                                            crps[j][:, lsl], op=Alu.mult)
                    nc.tensor.matmul(yps[:], ident, ymm[:],
                                     start=(j == POOLYJ[0]), stop=False)
                crs = {}
                for j in DVEYJ[:3]:
                    cr = pbc.tile([DI, CH], bf16, tag="bc")
                    crs[j] = cr
                    nc.tensor.matmul(cr[:], crep_lhsT[:, j * DI:(j + 1) * DI],
                                     XC[:, ysl], is_transpose=True)
                for i, j in enumerate(DVEYJ):
                    ymm = ypool.tile([DI, CH], bf16, tag="ymm")
                    nc.vector.tensor_tensor(ymm[:], Hs[j][:, ysl],
                                            crs.pop(j)[:], op=Alu.mult)
                    if i + 3 < len(DVEYJ):
                        j2 = DVEYJ[i + 3]
                        cr = pbc.tile([DI, CH], bf16, tag="bc")
                        crs[j2] = cr
                        nc.tensor.matmul(cr[:],
                                         crep_lhsT[:, j2 * DI:(j2 + 1) * DI],
                                         XC[:, ysl], is_transpose=True)
                    nc.tensor.matmul(yps[:], ident, ymm[:],
                                     start=False, stop=(i == len(DVEYJ) - 1))
                nc.scalar.activation(ysb[:, ysl], yps[:], Act.Copy)

            # segment boundaries (in 512-chunks): uneven so the tail is short
            SEGS = [(0, 3), (3, 6), (6, 8)]

            def emit_post(seg):
                c0, c1 = SEGS[seg]
                ssl = slice(c0 * CH, c1 * CH)
                nc.vector.tensor_add(XCD[:, ssl], YSUM[:, ssl], XCD[:, ssl])
                nc.vector.tensor_mul(YS[:, ssl], XCD[:, ssl], ZS[:, ssl])
                for ci in range(c0, c1):
                    sl = slice(ci * CH, (ci + 1) * CH)
                    op_ps = ps.tile([C, CH], f32, tag="mm")
                    nc.tensor.matmul(op_ps[:], op_lhsT, YS[:, sl],
                                     start=True, stop=False)
                    nc.tensor.matmul(op_ps[:], ident[0:C, 0:C], SEQ[:, sl],
                                     start=False, stop=True)
                    nc.scalar.activation(
                        OUTH[seg][:, (ci - c0) * CH:(ci - c0 + 1) * CH],
                        op_ps[:], Act.Copy)
                nc.sync.dma_start(out_d[:, ssl], OUTH[seg][:, 0:(c1 - c0) * CH])

            # stage B/C rows to DRAM (for the gpsimd states' broadcasts)
            for h in range(2):
                hsl = slice(h * 2048, (h + 1) * 2048)
                nc.sync.dma_start(bc_d[:, hsl], BCROWS[:, hsl])

            for seg, (c0, c1) in enumerate(SEGS):
                t0, t1 = c0 * CH, c1 * CH
                ssl = slice(t0, t1)
                slen = t1 - t0
                # broadcast B rows (and C rows) for pool states into SBUF
                brps = {}
                crps = {}
                for j in POOLJ:
                    brp = brcpool.tile([DI, slen], bf16, tag=f"brp{j}")
                    nc.sync.dma_start(
                        brp[:], bc_d[j:j + 1, ssl].to_broadcast((DI, slen)))
                    brps[j] = brp
                for j in POOLYJ:
                    crp = brcpool.tile([DI, slen], bf16, tag=f"crp{j}")
                    nc.sync.dma_start(
                        crp[:],
                        bc_d[8 + j:9 + j, ssl].to_broadcast((DI, slen)))
                    crps[j] = crp
                # scans for this segment
                for j in order:
                    dA = dapool.tile([DI, slen], f32, tag="dA")
                    nc.scalar.activation(dA[:], DT[:, ssl], Act.Exp,
                                         scale=a_sc[:, j:j + 1])
                    dbx = dbxpool.tile([DI, slen], bf16, tag="dbx")
                    if j in POOLJ:
                        nc.gpsimd.tensor_tensor(dbx[:], U[:, ssl],
                                                brps[j][:], op=Alu.mult)
                    else:
                        for ci in range(c0, c1):
                            bsl = slice(ci * CH, (ci + 1) * CH)
                            lsl = slice((ci - c0) * CH, (ci - c0 + 1) * CH)
                            br = pbc.tile([DI, CH], bf16, tag="bc")
                            nc.tensor.matmul(br[:],
                                             brep_lhsT[:, j * DI:(j + 1) * DI],
                                             XC[:, bsl], is_transpose=True)
                            nc.vector.tensor_tensor(dbx[:, lsl], U[:, bsl],
                                                    br[:], op=Alu.mult)
                    init = 0.0 if seg == 0 else Hs[j][:, t0 - 1:t0]
                    nc.vector.tensor_tensor_scan(Hs[j][:, ssl], dA[:], dbx[:],
                                                 init, op0=Alu.mult,
                                                 op1=Alu.add)
                # y for this segment
                for c in range(c0, c1):
                    y_chunk(c, crps, c0)
                nc.sync.dma_start(y_in_segs[seg][:], ysb[:, ssl])
                if sim:
                    nc.sync.dma_start(y_out_segs[seg][:], y_in_segs[seg][:])
                else:
                    nc.gpsimd.collective_compute(
                        "AllReduce", Alu.add, replica_groups=groups,
                        ins=[y_in_segs[seg].opt()],
                        outs=[y_out_segs[seg].opt()])
                nc.sync.dma_start(YSUM[:, ssl], y_out_segs[seg][:])
                # post of the PREVIOUS segment (its AR has landed by now)
                if seg >= 1:
                    emit_post(seg - 1)
            emit_post(len(SEGS) - 1)

    nc.compile()
    return nc


def _host_precompute(inp):
    import ml_dtypes
    f = lambda k: np.asarray(inp[k], np.float32)
    bf = lambda a: np.ascontiguousarray(a.astype(ml_dtypes.bfloat16))
    w1 = f("conv_w")[:, :, 0, 0]
    wh = f("dwh_w")[:, 0, :, 0]
    ww = f("dww_w")[:, 0, 0, :]
    taps = [
        w1 * (1.0 + wh[:, 1] + ww[:, 1])[None, :],   # center
        w1 * wh[:, 0][None, :],                       # up
        w1 * wh[:, 2][None, :],                       # down
        w1 * ww[:, 0][None, :],                       # left
        w1 * ww[:, 2][None, :],                       # right
    ]
    cw = np.concatenate([t.T for t in taps], axis=1)  # [64, 320]
    btot = f("conv_b") + w1 @ (f("dwh_b") + f("dww_b"))
    s_bn = f("bn_g") / np.sqrt(f("bn_v") + EPS)
    bn_bias = s_bn * (btot - f("bn_m")) + f("bn_b")
    ipw = f("in_proj_w")                               # [256, 64]
    ln_g = f("ln_g")
    ipx = ipw[:DI] * ln_g[None, :]                     # [128, 64]
    ipz = ipw[DI:] * ln_g[None, :]
    b_xm = ipw[:DI] @ f("ln_b")                        # [128]
    b_z = ipw[DI:] @ f("ln_b")
    cdw = f("convd_w")[:, 0, :]                        # [128, 4]
    # folded conv taps: Wk_lhsT[c, d] = ipx[d, c] * w_k[d]
    wk = np.concatenate([(ipx * cdw[:, k][:, None]).T for k in range(4)],
                        axis=1)                        # [64, 512]
    b_xc = cdw.sum(1) * b_xm + f("convd_b")
    # boundary corr for t in {0,1,2}: subtract (sum_{k<3-t} w_k) * b_xm
    corr = np.zeros((3, DI), np.float32)
    for t in range(3):
        corr[t] = -cdw[:, :3 - t].sum(1) * b_xm
    xpw = f("x_proj_w")                                # [36, 128]
    dtM = f("dt_proj_w") @ xpw[:DR]                    # [128, 128]
    a_full = -np.exp(np.asarray(inp["A_log"], np.float32))

    per_sigma = []
    for sg in range(2):
        s_lo = sg * NS
        cf32 = np.zeros((128, 16), np.float32)
        cf32[:C, 0] = s_bn
        cf32[:C, 1] = bn_bias
        cf32[:, 2] = b_z
        cf32[:, 3] = b_xc
        cf32[:, 4] = f("dt_proj_b")
        for j in range(NS):
            cf32[:, 5 + j] = a_full[:, s_lo + j]
        cf32[:, 13] = f("Dp")

        cbf = np.zeros((128, NCB), np.float32)
        cbf[:, 0:128] = np.eye(128, dtype=np.float32)
        cbf[:C, 128:448] = cw
        cbf[:C, 448:576] = ipz.T
        cbf[:C, 576:1088] = wk
        cbf[:, 1088:1216] = dtM.T
        for j in range(NS):
            s = s_lo + j
            cbf[:, 1216 + j * DI:1216 + (j + 1) * DI] = xpw[DR + s][:, None]
            cbf[:, 2240 + j * DI:2240 + (j + 1) * DI] = xpw[DR + DS + s][:, None]
        cbf[:, 3264:3328] = f("out_proj_w").T
        cbf[0:3, 3328:3456] = corr
        for j in range(NS):
            cbf[:, 3456 + j] = xpw[DR + s_lo + j]
            cbf[:, 3464 + j] = xpw[DR + DS + s_lo + j]
        per_sigma.append(dict(cf32=cf32, cbf=bf(cbf)))
    return {}, per_sigma


def _shift_images(xb):
    # 5 pre-shifted copies: ctr, up(reads h-1), dn(h+1), lf(w-1), rt(w+1)
    import ml_dtypes
    out = np.zeros((C, 5, H, W), np.float32)
    out[:, 0] = xb
    out[:, 1, 1:, :] = xb[:, :-1, :]
    out[:, 2, :-1, :] = xb[:, 1:, :]
    out[:, 3, :, 1:] = xb[:, :, :-1]
    out[:, 4, :, :-1] = xb[:, :, 1:]
    return np.ascontiguousarray(
        out.transpose(1, 0, 2, 3).reshape(5, C, L).transpose(1, 0, 2)
        .reshape(C, 5 * L).astype(ml_dtypes.bfloat16))


TRACE = False
LAST_EXEC_NS = None
LAST_TRACE_DIR = None


def kernel(**inputs):
    global LAST_EXEC_NS, LAST_TRACE_DIR
    from concourse.bass_utils import run_bass_kernel_spmd

    if "nc" not in _cached:
        _cached["nc"] = _build_program()
    nc = _cached["nc"]

    common, per_sigma = _host_precompute(inputs)
    x = np.asarray(inputs["x"], np.float32)
    in_maps = []
    for c in range(NCORES):
        b, sg = c // 2, c % 2
        m = dict(common)
        m.update(per_sigma[sg])
        m["ximgs"] = _shift_images(x[b])
        in_maps.append(m)

    kw = {}
    if TRACE:
        import tempfile
        LAST_TRACE_DIR = tempfile.mkdtemp(prefix="bass_trace_")
        kw = dict(trace=True, tmpdir=LAST_TRACE_DIR)
    r = run_bass_kernel_spmd(nc, in_maps, list(range(NCORES)), **kw)
    if r.exec_time_ns is not None:
        LAST_EXEC_NS = r.exec_time_ns
    res = r.results
    out = np.empty((B, C, H, W), np.float32)
    for b in range(B):
        out[b] = np.asarray(res[2 * b]["out_f"], np.float32).reshape(C, H, W)
    return out


# revision 3
# speedup vs baseline: 1.5717x; 1.0129x over previous
"""Trainium2 Bass kernel v4 for nn_DecoderBlock_Mamba.

Sharding: 8 cores = (batch b in 0..3) x (state-half sigma in {0,1}).
Each core runs the full per-image pipeline for its batch element but only 8 of
the 16 SSM states; partial y is AllReduce'd within core pairs.

Structure:
- causal conv1d folded into in_proj (4 shifted accumulated matmuls with
  host-precomputed diag(w_k) @ W weights; K=3 boundary-correction matmul)
- dt_proj @ x_proj_dt folded into one host matrix (rank-4 [128,128])
- b/c broadcasts via transpose-mode matmuls -> bf16 PSUM, dbx/y mults at
  DVE 2x; 3 states' dbx+scan and 2 states' y mult run on GPSIMD
- front conv + LayerNorm chunk-interleaved; per-group rstd via sqrt table
  (relu/square/sqrt all live in one act table); silus emitted after all
  sqrt-table ops so only ~5 act-table loads happen
- back half (dA/dbx/scan, y, AllReduce, post, out_proj) runs in 2
  L/2-segments, software-pipelined: y of segment 0 overlaps scans of
  segment 1, AllReduce latency hides under compute
- LN ssq reduces + HNP copies on GPSIMD during phase 1

Self-contained: hardcodes all shapes; no sibling imports.
"""
import numpy as np

C = 64
DI = 128
DS = 16
DR = 4
B = 4
H = 64
W = 64
L = H * W
NS = 8            # states per core
NCORES = 8
NCH = 8           # L chunks of 512
CH = 512
SEG = 2048
EPS = 1e-5
NPOOL = 4         # states whose dbx mult runs on gpsimd
NPOOLY = 3        # states whose y mult runs on gpsimd
NCB = 3472        # cbf columns

_cached = {}


def _build_program(sim=False):
    import concourse.bass as bass
    import concourse.bacc as bacc
    import concourse.mybir as mybir
    import concourse.tile as tile

    dt = mybir.dt
    f32 = dt.float32
    bf16 = dt.bfloat16
    Act = mybir.ActivationFunctionType
    Alu = mybir.AluOpType
    Axis = mybir.AxisListType

    nc = bacc.Bacc(None, target_bir_lowering=False)

    def din(name, shape, dtype=f32):
        return nc.dram_tensor(name, shape, dtype, kind="ExternalInput")

    ximgs_d = din("ximgs", [C, 5 * L], bf16)
    cf32_d = din("cf32", [128, 16])
    cbf_d = din("cbf", [128, NCB], bf16)

    out_d = nc.dram_tensor("out_f", [C, L], f32, kind="ExternalOutput")

    groups = [[0, 1], [2, 3], [4, 5], [6, 7]]
    # emission order of states: pool states first so their long chain starts
    # early
    order = [4, 0, 5, 1, 6, 2, 7, 3]

    with tile.TileContext(nc) as tc:
        with (
            tc.tile_pool(name="dram", bufs=1, space="DRAM") as dpool,
            tc.tile_pool(name="const", bufs=1) as cpool,
            tc.tile_pool(name="big", bufs=1) as bpool,
            tc.tile_pool(name="sm", bufs=2) as spool,
            tc.tile_pool(name="da", bufs=2) as dapool,
            tc.tile_pool(name="dbx", bufs=2) as dbxpool,
            tc.tile_pool(name="ymm", bufs=2) as ypool,
            tc.tile_pool(name="brc", bufs=1) as brcpool,
            tc.tile_pool(name="ps", bufs=2, space="PSUM") as ps,
            tc.tile_pool(name="bc", bufs=4, space="PSUM") as pbc,
            tc.tile_pool(name="psy", bufs=2, space="PSUM") as psy,
        ):
            # ---- constants (front-needed cols first) ----
            cf = cpool.tile([128, 16], f32)
            cb = cpool.tile([128, NCB], bf16)
            nc.sync.dma_start(cf[:], cf32_d[:])
            nc.sync.dma_start(cb[:, 0:448], cbf_d[:, 0:448])
            nc.sync.dma_start(cb[:, 448:NCB], cbf_d[:, 448:NCB])
            bn_s = cf[0:C, 0:1]
            bn_b = cf[0:C, 1:2]
            b_z = cf[:, 2:3]
            b_xc = cf[:, 3:4]
            dt_b = cf[:, 4:5]
            a_sc = cf[:, 5:13]
            Dp = cf[:, 13:14]
            ident = cb[:, 0:128]
            cw = cb[0:C, 128:448]
            ipz_lhsT = cb[0:C, 448:576]
            wk_lhsT = cb[0:C, 576:1088]       # 4 x [64,128]
            dtM_lhsT = cb[:, 1088:1216]
            brep_lhsT = cb[:, 1216:2240]      # 8 x [128,128]
            crep_lhsT = cb[:, 2240:3264]
            op_lhsT = cb[:, 3264:3328]
            corrT = cb[0:3, 3328:3456]        # [3,128]
            bc_lhsT = cb[:, 3456:3472]        # [128,16] B rows 0-7, C rows 8-15

            # ---- persistent activations ----
            SEQ = bpool.tile([C, L], bf16)
            HNP = bpool.tile([C, L + 3], bf16)     # ln-normed, 3 zero pad cols
            hnT = bpool.tile([128, 2048], bf16, name="hnT", tag="BCROWS")
            XC = bpool.tile([DI, L], bf16)
            DT = bpool.tile([DI, L], bf16)
            U = bpool.tile([DI, L], bf16)
            ZS = bpool.tile([DI, L], bf16)
            Hs = [bpool.tile([DI, L], bf16, name=f"H{j}", tag=f"H{j}")
                  for j in range(NS)]
            YSUM = bpool.tile([DI, L], bf16)
            SQ32 = bpool.tile([128, 2048], f32, name="SQ32", tag="SCR")
            esb = bpool.tile([DI, L], bf16, name="esb", tag="SCR")
            XCD = bpool.tile([DI, L], bf16, name="XCD", tag="SCR")
            YS = bpool.tile([DI, L], bf16, name="YS", tag="YS")
            ysb = bpool.tile([DI, L], bf16, name="ysb", tag="YS")
            BCROWS = bpool.tile([16, L], bf16, name="BCROWS", tag="BCROWS")
            MU = spool.tile([128, 32], f32, tag="MU")
            SUM32 = spool.tile([128, 32], f32, tag="SUM32")
            SSQ32 = spool.tile([128, 32], f32, tag="SSQ32")
            VAR = spool.tile([128, 32], f32, tag="VAR")
            SQV = spool.tile([128, 32], f32, tag="SQV")
            RSTD = spool.tile([128, 32], f32, tag="RSTD")

            IMGS = [bpool.tile([C, L], bf16, name=f"img{t}", tag=f"H{t}")
                    for t in range(5)]
            for h in range(4):
                for t in range(5):
                    nc.sync.dma_start(
                        IMGS[t][:, h * 1024:(h + 1) * 1024],
                        ximgs_d[:, t * L + h * 1024:t * L + (h + 1) * 1024])

            nc.gpsimd.memset(HNP[:, 0:3], 0.0)

            # Prime ACT's vector clock on the const DMAs (limited wait slots).
            warm = cpool.tile([128, 1], f32, tag="warm")
            nc.scalar.activation(warm[:], cf[:, 0:1], Act.Copy)
            warm2 = cpool.tile([128, 1], bf16, tag="warm2")
            nc.scalar.activation(warm2[:], cb[:, 0:1], Act.Copy)

            # ---- phase 1a: front conv + LayerNorm, chunk-interleaved ----
            # ACT functions here: Relu, Square, Sqrt, Copy (one table)
            for g in range(NCH):
                sl = slice(g * CH, (g + 1) * CH)
                pc = ps.tile([C, CH], f32, tag="mm")
                for tap in range(5):
                    nc.tensor.matmul(pc[:], cw[:, tap * C:(tap + 1) * C],
                                     IMGS[tap][:, sl],
                                     start=(tap == 0), stop=(tap == 4))
                nc.scalar.activation(SEQ[:, sl], pc[:],
                                     Act.Relu, bias=bn_b, scale=bn_s)
                g4 = slice(g * 4, (g + 1) * 4)
                tps4 = psy.tile([128, 4, C], bf16, tag="y")
                for k in range(4):
                    blk = g * 4 + k
                    nc.tensor.transpose(tps4[:, k, :],
                                        SEQ[:, blk * 128:(blk + 1) * 128],
                                        ident[0:C, 0:C])
                nc.vector.tensor_reduce(SUM32[:, g4], tps4[:], Axis.X, Alu.add)
                nc.scalar.activation(SQ32[:, g * 256:(g + 1) * 256], tps4[:],
                                     Act.Square)
                nc.vector.tensor_reduce(
                    SSQ32[:, g4],
                    SQ32[:, g * 256:(g + 1) * 256].rearrange(
                        "p (b c) -> p b c", b=4),
                    Axis.X, Alu.add)
                nc.vector.tensor_scalar_mul(MU[:, g4], SUM32[:, g4], 1.0 / C)
                MUSQ = spool.tile([128, 4], f32, tag="MUSQ")
                nc.vector.tensor_mul(MUSQ[:], MU[:, g4], MU[:, g4])
                nc.vector.tensor_scalar(VAR[:, g4], SSQ32[:, g4], 1.0 / C, EPS,
                                        op0=Alu.mult, op1=Alu.add)
                nc.vector.tensor_tensor(VAR[:, g4], VAR[:, g4], MUSQ[:],
                                        op=Alu.subtract)
                nc.scalar.activation(SQV[:, g4], VAR[:, g4], Act.Sqrt)
                nc.vector.reciprocal(RSTD[:, g4], SQV[:, g4])
                for k in range(4):
                    blk = g * 4 + k
                    nc.vector.tensor_scalar(
                        hnT[:, blk * C:(blk + 1) * C], tps4[:, k, :],
                        MU[:, blk:blk + 1], RSTD[:, blk:blk + 1],
                        op0=Alu.subtract, op1=Alu.mult)
                tb4 = pbc.tile([C, 4, 128], bf16, tag="bc")
                for k in range(4):
                    blk = g * 4 + k
                    nc.tensor.transpose(tb4[:, k, :],
                                        hnT[:, blk * C:(blk + 1) * C],
                                        ident)
                nc.vector.tensor_copy(HNP[:, 3 + g * CH:3 + (g + 1) * CH],
                                      tb4[:].rearrange("p a b -> p (a b)"))

            # ---- phase 1b: in_proj + folded conv1d + z (silu table) ----
            # ones gate derived from the last sqrt: forces the scheduler to
            # place every silu after the sqrt-table region (avoids act-table
            # thrash; numerically scale=1.0 exactly)
            ones_t = spool.tile([128, 1], f32, tag="ones")
            nc.vector.tensor_scalar(ones_t[:], SQV[:, 31:32], 0.0, 1.0,
                                    op0=Alu.mult, op1=Alu.add)
            for chi in range(NCH):
                sl = slice(chi * CH, (chi + 1) * CH)
                xc_ps = ps.tile([DI, CH], f32, tag="mm")
                for k in range(4):
                    nc.tensor.matmul(xc_ps[:], wk_lhsT[:, k * 128:(k + 1) * 128],
                                     HNP[:, k + chi * CH:k + chi * CH + CH],
                                     start=(k == 0),
                                     stop=(k == 3 and chi != 0))
                if chi == 0:
                    nc.tensor.matmul(xc_ps[:, 0:3], corrT,
                                     ident[0:3, 0:3],
                                     start=False, stop=True)
                nc.scalar.activation(XC[:, sl], xc_ps[:], Act.Silu, bias=b_xc,
                                     scale=ones_t)
                z_ps = ps.tile([DI, CH], f32, tag="mm")
                nc.tensor.matmul(z_ps[:], ipz_lhsT,
                                 HNP[:, 3 + chi * CH:3 + (chi + 1) * CH],
                                 start=True, stop=True)
                nc.scalar.activation(ZS[:, sl], z_ps[:], Act.Silu, bias=b_z,
                                     scale=ones_t)

            # ---- phase 1c: dt path (exp chunks, then chunked ln + U) ----
            for chi in range(NCH):
                sl = slice(chi * CH, (chi + 1) * CH)
                dt_ps = ps.tile([DI, CH], f32, tag="mm")
                nc.tensor.matmul(dt_ps[:], dtM_lhsT, XC[:, sl],
                                 start=True, stop=True)
                nc.scalar.activation(esb[:, sl], dt_ps[:], Act.Exp, bias=dt_b)
                nc.vector.tensor_scalar_mul(XCD[:, sl], XC[:, sl], Dp)
                bc_ps = pbc.tile([16, CH], bf16, tag="bc")
                nc.tensor.matmul(bc_ps[:], bc_lhsT, XC[:, sl],
                                 is_transpose=True)
                nc.scalar.activation(BCROWS[:, sl], bc_ps[:], Act.Copy)
            for chi in range(NCH):
                sl = slice(chi * CH, (chi + 1) * CH)
                nc.scalar.activation(DT[:, sl], esb[:, sl], Act.Ln, bias=1.0)
                nc.vector.tensor_mul(U[:, sl], DT[:, sl], XC[:, sl])

            # ---- phase 2: per-segment scan + y + AR + post + out ----
            y_in_segs = [dpool.tile([DI, n * CH], bf16, name=f"y_in{i}", tag=f"yin{i}")
                         for i, n in enumerate((3, 3, 2))]
            y_out_segs = [dpool.tile([DI, n * CH], bf16, name=f"y_out{i}", tag=f"yout{i}")
                          for i, n in enumerate((3, 3, 2))]
            bc_d = dpool.tile([16, L], bf16, tag="bcd")
            OUTH = [bpool.tile([C, 1536], f32, name=f"outh{h}",
                                tag=f"OUTH{h % 2}") for h in range(3)]
            POOLJ = list(range(NS - NPOOL, NS))     # dbx+scan on gpsimd
            POOLYJ = list(range(NS - NPOOLY, NS))   # y mult on gpsimd
            DVEYJ = [j for j in range(NS) if j not in POOLYJ]

            def y_chunk(c, crps, c0):
                ysl = slice(c * CH, (c + 1) * CH)
                lsl = slice((c - c0) * CH, (c - c0 + 1) * CH)
                yps = psy.tile([DI, CH], f32, tag="y")
                # pool states first: their ymm comes from SBUF crp tiles and
                # can start immediately, overlapping the cr matmuls below
                for i, j in enumerate(POOLYJ):
                    ymm = ypool.tile([DI, CH], bf16, tag="ymmp")
                    nc.gpsimd.tensor_tensor(ymm[:], Hs[j][:, ysl],
                                            crps[j][:, lsl], op=Alu.mult)
                    nc.tensor.matmul(yps[:], ident, ymm[:],
                                     start=(j == POOLYJ[0]), stop=False)
                crs = {}
                for j in DVEYJ[:3]:
                    cr = pbc.tile([DI, CH], bf16, tag="bc")
                    crs[j] = cr
                    nc.tensor.matmul(cr[:], crep_lhsT[:, j * DI:(j + 1) * DI],
                                     XC[:, ysl], is_transpose=True)
                for i, j in enumerate(DVEYJ):
                    ymm = ypool.tile([DI, CH], bf16, tag="ymm")
                    nc.vector.tensor_tensor(ymm[:], Hs[j][:, ysl],
                                            crs.pop(j)[:], op=Alu.mult)
                    if i + 3 < len(DVEYJ):
                        j2 = DVEYJ[i + 3]
                        cr = pbc.tile([DI, CH], bf16, tag="bc")
                        crs[j2] = cr
                        nc.tensor.matmul(cr[:],
                                         crep_lhsT[:, j2 * DI:(j2 + 1) * DI],
                                         XC[:, ysl], is_transpose=True)
                    nc.tensor.matmul(yps[:], ident, ymm[:],
                                     start=False, stop=(i == len(DVEYJ) - 1))
                nc.scalar.activation(ysb[:, ysl], yps[:], Act.Copy)

            # segment boundaries (in 512-chunks): uneven so the tail is short
            SEGS = [(0, 3), (3, 6), (6, 8)]

            def emit_post(seg):
                c0, c1 = SEGS[seg]
                ssl = slice(c0 * CH, c1 * CH)
                nc.vector.tensor_add(XCD[:, ssl], YSUM[:, ssl], XCD[:, ssl])
                nc.vector.tensor_mul(YS[:, ssl], XCD[:, ssl], ZS[:, ssl])
                for ci in range(c0, c1):
                    sl = slice(ci * CH, (ci + 1) * CH)
                    op_ps = ps.tile([C, CH], f32, tag="mm")
                    nc.tensor.matmul(op_ps[:], op_lhsT, YS[:, sl],
                                     start=True, stop=False)
                    nc.tensor.matmul(op_ps[:], ident[0:C, 0:C], SEQ[:, sl],
                                     start=False, stop=True)
                    nc.scalar.activation(
                        OUTH[seg][:, (ci - c0) * CH:(ci - c0 + 1) * CH],
                        op_ps[:], Act.Copy)
                nc.sync.dma_start(out_d[:, ssl], OUTH[seg][:, 0:(c1 - c0) * CH])

            # stage B/C rows to DRAM (for the gpsimd states' broadcasts)
            for h in range(2):
                hsl = slice(h * 2048, (h + 1) * 2048)
                nc.sync.dma_start(bc_d[:, hsl], BCROWS[:, hsl])

            for seg, (c0, c1) in enumerate(SEGS):
                t0, t1 = c0 * CH, c1 * CH
                ssl = slice(t0, t1)
                slen = t1 - t0
                # broadcast B rows (and C rows) for pool states into SBUF
                brps = {}
                crps = {}
                for j in POOLJ:
                    brp = brcpool.tile([DI, slen], bf16, tag=f"brp{j}")
                    nc.sync.dma_start(
                        brp[:], bc_d[j:j + 1, ssl].to_broadcast((DI, slen)))
                    brps[j] = brp
                for j in POOLYJ:
                    crp = brcpool.tile([DI, slen], bf16, tag=f"crp{j}")
                    nc.sync.dma_start(
                        crp[:],
                        bc_d[8 + j:9 + j, ssl].to_broadcast((DI, slen)))
                    crps[j] = crp
                # scans for this segment
                for j in order:
                    dA = dapool.tile([DI, slen], f32, tag="dA")
                    nc.scalar.activation(dA[:], DT[:, ssl], Act.Exp,
                                         scale=a_sc[:, j:j + 1])
                    dbx = dbxpool.tile([DI, slen], bf16, tag="dbx")
                    if j in POOLJ:
                        nc.gpsimd.tensor_tensor(dbx[:], U[:, ssl],
                                                brps[j][:], op=Alu.mult)
                    else:
                        for ci in range(c0, c1):
                            bsl = slice(ci * CH, (ci + 1) * CH)
                            lsl = slice((ci - c0) * CH, (ci - c0 + 1) * CH)
                            br = pbc.tile([DI, CH], bf16, tag="bc")
                            nc.tensor.matmul(br[:],
                                             brep_lhsT[:, j * DI:(j + 1) * DI],
                                             XC[:, bsl], is_transpose=True)
                            nc.vector.tensor_tensor(dbx[:, lsl], U[:, bsl],
                                                    br[:], op=Alu.mult)
                    init = 0.0 if seg == 0 else Hs[j][:, t0 - 1:t0]
                    nc.vector.tensor_tensor_scan(Hs[j][:, ssl], dA[:], dbx[:],
                                                 init, op0=Alu.mult,
                                                 op1=Alu.add)
                # y for this segment
                for c in range(c0, c1):
                    y_chunk(c, crps, c0)
                nc.sync.dma_start(y_in_segs[seg][:], ysb[:, ssl])
                if sim:
                    nc.sync.dma_start(y_out_segs[seg][:], y_in_segs[seg][:])
                else:
                    nc.gpsimd.collective_compute(
                        "AllReduce", Alu.add, replica_groups=groups,
                        ins=[y_in_segs[seg].opt()],
                        outs=[y_out_segs[seg].opt()])
                nc.sync.dma_start(YSUM[:, ssl], y_out_segs[seg][:])
                # post of the PREVIOUS segment (its AR has landed by now)
                if seg >= 1:
                    emit_post(seg - 1)
            emit_post(len(SEGS) - 1)

    nc.compile()
    return nc


def _host_precompute(inp):
    import ml_dtypes
    f = lambda k: np.asarray(inp[k], np.float32)
    bf = lambda a: np.ascontiguousarray(a.astype(ml_dtypes.bfloat16))
    w1 = f("conv_w")[:, :, 0, 0]
    wh = f("dwh_w")[:, 0, :, 0]
    ww = f("dww_w")[:, 0, 0, :]
    taps = [
        w1 * (1.0 + wh[:, 1] + ww[:, 1])[None, :],   # center
        w1 * wh[:, 0][None, :],                       # up
        w1 * wh[:, 2][None, :],                       # down
        w1 * ww[:, 0][None, :],                       # left
        w1 * ww[:, 2][None, :],                       # right
    ]
    cw = np.concatenate([t.T for t in taps], axis=1)  # [64, 320]
    btot = f("conv_b") + w1 @ (f("dwh_b") + f("dww_b"))
    s_bn = f("bn_g") / np.sqrt(f("bn_v") + EPS)
    bn_bias = s_bn * (btot - f("bn_m")) + f("bn_b")
    ipw = f("in_proj_w")                               # [256, 64]
    ln_g = f("ln_g")
    ipx = ipw[:DI] * ln_g[None, :]                     # [128, 64]
    ipz = ipw[DI:] * ln_g[None, :]
    b_xm = ipw[:DI] @ f("ln_b")                        # [128]
    b_z = ipw[DI:] @ f("ln_b")
    cdw = f("convd_w")[:, 0, :]                        # [128, 4]
    # folded conv taps: Wk_lhsT[c, d] = ipx[d, c] * w_k[d]
    wk = np.concatenate([(ipx * cdw[:, k][:, None]).T for k in range(4)],
                        axis=1)                        # [64, 512]
    b_xc = cdw.sum(1) * b_xm + f("convd_b")
    # boundary corr for t in {0,1,2}: subtract (sum_{k<3-t} w_k) * b_xm
    corr = np.zeros((3, DI), np.float32)
    for t in range(3):
        corr[t] = -cdw[:, :3 - t].sum(1) * b_xm
    xpw = f("x_proj_w")                                # [36, 128]
    dtM = f("dt_proj_w") @ xpw[:DR]                    # [128, 128]
    a_full = -np.exp(np.asarray(inp["A_log"], np.float32))

    per_sigma = []
    for sg in range(2):
        s_lo = sg * NS
        cf32 = np.zeros((128, 16), np.float32)
        cf32[:C, 0] = s_bn
        cf32[:C, 1] = bn_bias
        cf32[:, 2] = b_z
        cf32[:, 3] = b_xc
        cf32[:, 4] = f("dt_proj_b")
        for j in range(NS):
            cf32[:, 5 + j] = a_full[:, s_lo + j]
        cf32[:, 13] = f("Dp")

        cbf = np.zeros((128, NCB), np.float32)
        cbf[:, 0:128] = np.eye(128, dtype=np.float32)
        cbf[:C, 128:448] = cw
        cbf[:C, 448:576] = ipz.T
        cbf[:C, 576:1088] = wk
        cbf[:, 1088:1216] = dtM.T
        for j in range(NS):
            s = s_lo + j
            cbf[:, 1216 + j * DI:1216 + (j + 1) * DI] = xpw[DR + s][:, None]
            cbf[:, 2240 + j * DI:2240 + (j + 1) * DI] = xpw[DR + DS + s][:, None]
        cbf[:, 3264:3328] = f("out_proj_w").T
        cbf[0:3, 3328:3456] = corr
        for j in range(NS):
            cbf[:, 3456 + j] = xpw[DR + s_lo + j]
            cbf[:, 3464 + j] = xpw[DR + DS + s_lo + j]
        per_sigma.append(dict(cf32=cf32, cbf=bf(cbf)))
    return {}, per_sigma


def _shift_images(xb):
    # 5 pre-shifted copies: ctr, up(reads h-1), dn(h+1), lf(w-1), rt(w+1)
    import ml_dtypes
    out = np.zeros((C, 5, H, W), np.float32)
    out[:, 0] = xb
    out[:, 1, 1:, :] = xb[:, :-1, :]
    out[:, 2, :-1, :] = xb[:, 1:, :]
    out[:, 3, :, 1:] = xb[:, :, :-1]
    out[:, 4, :, :-1] = xb[:, :, 1:]
    return np.ascontiguousarray(
        out.transpose(1, 0, 2, 3).reshape(5, C, L).transpose(1, 0, 2)
        .reshape(C, 5 * L).astype(ml_dtypes.bfloat16))


TRACE = False
LAST_EXEC_NS = None
LAST_TRACE_DIR = None


def kernel(**inputs):
    global LAST_EXEC_NS, LAST_TRACE_DIR
    from concourse.bass_utils import run_bass_kernel_spmd

    if "nc" not in _cached:
        _cached["nc"] = _build_program()
    nc = _cached["nc"]

    common, per_sigma = _host_precompute(inputs)
    x = np.asarray(inputs["x"], np.float32)
    in_maps = []
    for c in range(NCORES):
        b, sg = c // 2, c % 2
        m = dict(common)
        m.update(per_sigma[sg])
        m["ximgs"] = _shift_images(x[b])
        in_maps.append(m)

    kw = {}
    if TRACE:
        import tempfile
        LAST_TRACE_DIR = tempfile.mkdtemp(prefix="bass_trace_")
        kw = dict(trace=True, tmpdir=LAST_TRACE_DIR)
    r = run_bass_kernel_spmd(nc, in_maps, list(range(NCORES)), **kw)
    if r.exec_time_ns is not None:
        LAST_EXEC_NS = r.exec_time_ns
    res = r.results
    out = np.empty((B, C, H, W), np.float32)
    for b in range(B):
        out[b] = np.asarray(res[2 * b]["out_f"], np.float32).reshape(C, H, W)
    return out


# revision 4
# speedup vs baseline: 1.6246x; 1.0337x over previous
"""Trainium2 Bass kernel v4 for nn_DecoderBlock_Mamba.

Sharding: 8 cores = (batch b in 0..3) x (state-half sigma in {0,1}).
Each core runs the full per-image pipeline for its batch element but only 8 of
the 16 SSM states; partial y is AllReduce'd within core pairs.

Structure:
- causal conv1d folded into in_proj (4 shifted accumulated matmuls with
  host-precomputed diag(w_k) @ W weights; K=3 boundary-correction matmul)
- dt_proj @ x_proj_dt folded into one host matrix (rank-4 [128,128])
- b/c broadcasts via transpose-mode matmuls -> bf16 PSUM, dbx/y mults at
  DVE 2x; 3 states' dbx+scan and 2 states' y mult run on GPSIMD
- front conv + LayerNorm chunk-interleaved; per-group rstd via sqrt table
  (relu/square/sqrt all live in one act table); silus emitted after all
  sqrt-table ops so only ~5 act-table loads happen
- back half (dA/dbx/scan, y, AllReduce, post, out_proj) runs in 2
  L/2-segments, software-pipelined: y of segment 0 overlaps scans of
  segment 1, AllReduce latency hides under compute
- LN ssq reduces + HNP copies on GPSIMD during phase 1

Self-contained: hardcodes all shapes; no sibling imports.
"""
import numpy as np

C = 64
DI = 128
DS = 16
DR = 4
B = 4
H = 64
W = 64
L = H * W
NS = 8            # states per core
NCORES = 8
NCH = 8           # L chunks of 512
CH = 512
SEG = 2048
EPS = 1e-5
NPOOL = 4         # states whose dbx mult runs on gpsimd
NPOOLY = 3        # states whose y mult runs on gpsimd
NCB = 3472        # cbf columns

_cached = {}


def _build_program(sim=False):
    import concourse.bass as bass
    import concourse.bacc as bacc
    import concourse.mybir as mybir
    import concourse.tile as tile

    dt = mybir.dt
    f32 = dt.float32
    bf16 = dt.bfloat16
    Act = mybir.ActivationFunctionType
    Alu = mybir.AluOpType
    Axis = mybir.AxisListType

    nc = bacc.Bacc(None, target_bir_lowering=False)

    def din(name, shape, dtype=f32):
        return nc.dram_tensor(name, shape, dtype, kind="ExternalInput")

    ximgs_d = din("ximgs", [C, 5 * L], bf16)
    cf32_d = din("cf32", [128, 16])
    cbf_d = din("cbf", [128, NCB], bf16)

    out_d = nc.dram_tensor("out_f", [C, L], f32, kind="ExternalOutput")

    groups = [[0, 1], [2, 3], [4, 5], [6, 7]]
    # emission order of states: pool states first so their long chain starts
    # early
    order = [5, 4, 0, 6, 1, 7, 2, 3]

    with tile.TileContext(nc) as tc:
        with (
            tc.tile_pool(name="dram", bufs=1, space="DRAM") as dpool,
            tc.tile_pool(name="const", bufs=1) as cpool,
            tc.tile_pool(name="big", bufs=1) as bpool,
            tc.tile_pool(name="sm", bufs=2) as spool,
            tc.tile_pool(name="da", bufs=2) as dapool,
            tc.tile_pool(name="dbx", bufs=2) as dbxpool,
            tc.tile_pool(name="ymm", bufs=2) as ypool,
            tc.tile_pool(name="brc", bufs=1) as brcpool,
            tc.tile_pool(name="ps", bufs=2, space="PSUM") as ps,
            tc.tile_pool(name="bc", bufs=4, space="PSUM") as pbc,
            tc.tile_pool(name="psy", bufs=2, space="PSUM") as psy,
        ):
            # ---- constants (front-needed cols first) ----
            cf = cpool.tile([128, 16], f32)
            cb = cpool.tile([128, NCB], bf16)
            nc.sync.dma_start(cf[:], cf32_d[:])
            nc.sync.dma_start(cb[:, 0:448], cbf_d[:, 0:448])
            nc.sync.dma_start(cb[:, 448:NCB], cbf_d[:, 448:NCB])
            bn_s = cf[0:C, 0:1]
            bn_b = cf[0:C, 1:2]
            b_z = cf[:, 2:3]
            b_xc = cf[:, 3:4]
            dt_b = cf[:, 4:5]
            a_sc = cf[:, 5:13]
            Dp = cf[:, 13:14]
            ident = cb[:, 0:128]
            cw = cb[0:C, 128:448]
            ipz_lhsT = cb[0:C, 448:576]
            wk_lhsT = cb[0:C, 576:1088]       # 4 x [64,128]
            dtM_lhsT = cb[:, 1088:1216]
            brep_lhsT = cb[:, 1216:2240]      # 8 x [128,128]
            crep_lhsT = cb[:, 2240:3264]
            op_lhsT = cb[:, 3264:3328]
            corrT = cb[0:3, 3328:3456]        # [3,128]
            bc_lhsT = cb[:, 3456:3472]        # [128,16] B rows 0-7, C rows 8-15

            # ---- persistent activations ----
            SEQ = bpool.tile([C, L], bf16)
            HNP = bpool.tile([C, L + 3], bf16)     # ln-normed, 3 zero pad cols
            hnT = bpool.tile([128, 2048], bf16, name="hnT", tag="BCROWS")
            XC = bpool.tile([DI, L], bf16)
            DT = bpool.tile([DI, L], bf16)
            U = bpool.tile([DI, L], bf16)
            ZS = bpool.tile([DI, L], bf16)
            Hs = [bpool.tile([DI, L], bf16, name=f"H{j}", tag=f"H{j}")
                  for j in range(NS)]
            YSUM = bpool.tile([DI, L], bf16)
            SQ32 = bpool.tile([128, 2048], f32, name="SQ32", tag="SCR")
            esb = bpool.tile([DI, L], bf16, name="esb", tag="SCR")
            XCD = bpool.tile([DI, L], bf16, name="XCD", tag="SCR")
            YS = bpool.tile([DI, L], bf16, name="YS", tag="YS")
            ysb = bpool.tile([DI, L], bf16, name="ysb", tag="YS")
            BCROWS = bpool.tile([16, L], bf16, name="BCROWS", tag="BCROWS")
            MU = spool.tile([128, 32], f32, tag="MU")
            SUM32 = spool.tile([128, 32], f32, tag="SUM32")
            SSQ32 = spool.tile([128, 32], f32, tag="SSQ32")
            VAR = spool.tile([128, 32], f32, tag="VAR")
            SQV = spool.tile([128, 32], f32, tag="SQV")
            RSTD = spool.tile([128, 32], f32, tag="RSTD")

            IMGS = [bpool.tile([C, L], bf16, name=f"img{t}", tag=f"H{t}")
                    for t in range(5)]
            for h in range(4):
                for t in range(5):
                    nc.sync.dma_start(
                        IMGS[t][:, h * 1024:(h + 1) * 1024],
                        ximgs_d[:, t * L + h * 1024:t * L + (h + 1) * 1024])

            nc.gpsimd.memset(HNP[:, 0:3], 0.0)

            # Prime ACT's vector clock on the const DMAs (limited wait slots).
            warm = cpool.tile([128, 1], f32, tag="warm")
            nc.scalar.activation(warm[:], cf[:, 0:1], Act.Copy)
            warm2 = cpool.tile([128, 1], bf16, tag="warm2")
            nc.scalar.activation(warm2[:], cb[:, 0:1], Act.Copy)

            # ---- phase 1a: front conv + LayerNorm, chunk-interleaved ----
            # ACT functions here: Relu, Square, Sqrt, Copy (one table)
            for g in range(NCH):
                sl = slice(g * CH, (g + 1) * CH)
                pc = ps.tile([C, CH], f32, tag="mm")
                for tap in range(5):
                    nc.tensor.matmul(pc[:], cw[:, tap * C:(tap + 1) * C],
                                     IMGS[tap][:, sl],
                                     start=(tap == 0), stop=(tap == 4))
                nc.scalar.activation(SEQ[:, sl], pc[:],
                                     Act.Relu, bias=bn_b, scale=bn_s)
                g4 = slice(g * 4, (g + 1) * 4)
                tps4 = psy.tile([128, 4, C], bf16, tag="y")
                for k in range(4):
                    blk = g * 4 + k
                    nc.tensor.transpose(tps4[:, k, :],
                                        SEQ[:, blk * 128:(blk + 1) * 128],
                                        ident[0:C, 0:C])
                nc.vector.tensor_reduce(SUM32[:, g4], tps4[:], Axis.X, Alu.add)
                nc.scalar.activation(SQ32[:, g * 256:(g + 1) * 256], tps4[:],
                                     Act.Square)
                nc.vector.tensor_reduce(
                    SSQ32[:, g4],
                    SQ32[:, g * 256:(g + 1) * 256].rearrange(
                        "p (b c) -> p b c", b=4),
                    Axis.X, Alu.add)
                nc.vector.tensor_scalar_mul(MU[:, g4], SUM32[:, g4], 1.0 / C)
                MUSQ = spool.tile([128, 4], f32, tag="MUSQ")
                nc.vector.tensor_mul(MUSQ[:], MU[:, g4], MU[:, g4])
                nc.vector.tensor_scalar(VAR[:, g4], SSQ32[:, g4], 1.0 / C, EPS,
                                        op0=Alu.mult, op1=Alu.add)
                nc.vector.tensor_tensor(VAR[:, g4], VAR[:, g4], MUSQ[:],
                                        op=Alu.subtract)
                nc.scalar.activation(SQV[:, g4], VAR[:, g4], Act.Sqrt)
                nc.vector.reciprocal(RSTD[:, g4], SQV[:, g4])
                for k in range(4):
                    blk = g * 4 + k
                    nc.vector.tensor_scalar(
                        hnT[:, blk * C:(blk + 1) * C], tps4[:, k, :],
                        MU[:, blk:blk + 1], RSTD[:, blk:blk + 1],
                        op0=Alu.subtract, op1=Alu.mult)
                tb4 = pbc.tile([C, 4, 128], bf16, tag="bc")
                for k in range(4):
                    blk = g * 4 + k
                    nc.tensor.transpose(tb4[:, k, :],
                                        hnT[:, blk * C:(blk + 1) * C],
                                        ident)
                nc.vector.tensor_copy(HNP[:, 3 + g * CH:3 + (g + 1) * CH],
                                      tb4[:].rearrange("p a b -> p (a b)"))

            # ---- phase 1b: in_proj + folded conv1d + z (silu table) ----
            # ones gate derived from the last sqrt: forces the scheduler to
            # place every silu after the sqrt-table region (avoids act-table
            # thrash; numerically scale=1.0 exactly)
            ones_t = spool.tile([128, 1], f32, tag="ones")
            nc.vector.tensor_scalar(ones_t[:], SQV[:, 31:32], 0.0, 1.0,
                                    op0=Alu.mult, op1=Alu.add)
            for chi in range(NCH):
                sl = slice(chi * CH, (chi + 1) * CH)
                xc_ps = ps.tile([DI, CH], f32, tag="mm")
                for k in range(4):
                    nc.tensor.matmul(xc_ps[:], wk_lhsT[:, k * 128:(k + 1) * 128],
                                     HNP[:, k + chi * CH:k + chi * CH + CH],
                                     start=(k == 0),
                                     stop=(k == 3 and chi != 0))
                if chi == 0:
                    nc.tensor.matmul(xc_ps[:, 0:3], corrT,
                                     ident[0:3, 0:3],
                                     start=False, stop=True)
                nc.scalar.activation(XC[:, sl], xc_ps[:], Act.Silu, bias=b_xc,
                                     scale=ones_t)
                z_ps = ps.tile([DI, CH], f32, tag="mm")
                nc.tensor.matmul(z_ps[:], ipz_lhsT,
                                 HNP[:, 3 + chi * CH:3 + (chi + 1) * CH],
                                 start=True, stop=True)
                nc.scalar.activation(ZS[:, sl], z_ps[:], Act.Silu, bias=b_z,
                                     scale=ones_t)

            # ---- phase 1c: dt path (exp chunks, then chunked ln + U) ----
            for chi in range(NCH):
                sl = slice(chi * CH, (chi + 1) * CH)
                dt_ps = ps.tile([DI, CH], f32, tag="mm")
                nc.tensor.matmul(dt_ps[:], dtM_lhsT, XC[:, sl],
                                 start=True, stop=True)
                nc.scalar.activation(esb[:, sl], dt_ps[:], Act.Exp, bias=dt_b)
                nc.vector.tensor_scalar_mul(XCD[:, sl], XC[:, sl], Dp)
                bc_ps = pbc.tile([16, CH], bf16, tag="bc")
                nc.tensor.matmul(bc_ps[:], bc_lhsT, XC[:, sl],
                                 is_transpose=True)
                nc.scalar.activation(BCROWS[:, sl], bc_ps[:], Act.Copy)
            for chi in range(NCH):
                sl = slice(chi * CH, (chi + 1) * CH)
                nc.scalar.activation(DT[:, sl], esb[:, sl], Act.Ln, bias=1.0)
                nc.vector.tensor_mul(U[:, sl], DT[:, sl], XC[:, sl])

            # ---- phase 2: per-segment scan + y + AR + post + out ----
            y_in_segs = [dpool.tile([DI, n * CH], bf16, name=f"y_in{i}", tag=f"yin{i}")
                         for i, n in enumerate((3, 3, 2))]
            y_out_segs = [dpool.tile([DI, n * CH], bf16, name=f"y_out{i}", tag=f"yout{i}")
                          for i, n in enumerate((3, 3, 2))]
            bc_d = dpool.tile([16, L], bf16, tag="bcd")
            OUTH = [bpool.tile([C, 1536], f32, name=f"outh{h}",
                                tag=f"OUTH{h % 2}") for h in range(3)]
            POOLJ = list(range(NS - NPOOL, NS))     # dbx+scan on gpsimd
            POOLYJ = list(range(NS - NPOOLY, NS))   # y mult on gpsimd
            DVEYJ = [j for j in range(NS) if j not in POOLYJ]

            def y_chunk(c, crps, c0):
                ysl = slice(c * CH, (c + 1) * CH)
                lsl = slice((c - c0) * CH, (c - c0 + 1) * CH)
                yps = psy.tile([DI, CH], f32, tag="y")
                # pool states first: their ymm comes from SBUF crp tiles and
                # can start immediately, overlapping the cr matmuls below
                for i, j in enumerate(POOLYJ):
                    ymm = ypool.tile([DI, CH], bf16, tag="ymmp")
                    nc.gpsimd.tensor_tensor(ymm[:], Hs[j][:, ysl],
                                            crps[j][:, lsl], op=Alu.mult)
                    nc.tensor.matmul(yps[:], ident, ymm[:],
                                     start=(j == POOLYJ[0]), stop=False)
                crs = {}
                for j in DVEYJ[:3]:
                    cr = pbc.tile([DI, CH], bf16, tag="bc")
                    crs[j] = cr
                    nc.tensor.matmul(cr[:], crep_lhsT[:, j * DI:(j + 1) * DI],
                                     XC[:, ysl], is_transpose=True)
                for i, j in enumerate(DVEYJ):
                    ymm = ypool.tile([DI, CH], bf16, tag="ymm")
                    nc.vector.tensor_tensor(ymm[:], Hs[j][:, ysl],
                                            crs.pop(j)[:], op=Alu.mult)
                    if i + 3 < len(DVEYJ):
                        j2 = DVEYJ[i + 3]
                        cr = pbc.tile([DI, CH], bf16, tag="bc")
                        crs[j2] = cr
                        nc.tensor.matmul(cr[:],
                                         crep_lhsT[:, j2 * DI:(j2 + 1) * DI],
                                         XC[:, ysl], is_transpose=True)
                    nc.tensor.matmul(yps[:], ident, ymm[:],
                                     start=False, stop=(i == len(DVEYJ) - 1))
                nc.scalar.activation(ysb[:, ysl], yps[:], Act.Copy)

            # segment boundaries (in 512-chunks): uneven so the tail is short
            SEGS = [(0, 3), (3, 6), (6, 8)]

            def emit_post(seg):
                c0, c1 = SEGS[seg]
                ssl = slice(c0 * CH, c1 * CH)
                nc.vector.tensor_add(XCD[:, ssl], YSUM[:, ssl], XCD[:, ssl])
                nc.vector.tensor_mul(YS[:, ssl], XCD[:, ssl], ZS[:, ssl])
                for ci in range(c0, c1):
                    sl = slice(ci * CH, (ci + 1) * CH)
                    op_ps = ps.tile([C, CH], f32, tag="mm")
                    nc.tensor.matmul(op_ps[:], op_lhsT, YS[:, sl],
                                     start=True, stop=False)
                    nc.tensor.matmul(op_ps[:], ident[0:C, 0:C], SEQ[:, sl],
                                     start=False, stop=True)
                    nc.scalar.activation(
                        OUTH[seg][:, (ci - c0) * CH:(ci - c0 + 1) * CH],
                        op_ps[:], Act.Copy)
                nc.sync.dma_start(out_d[:, ssl], OUTH[seg][:, 0:(c1 - c0) * CH])

            # stage B/C rows to DRAM (for the gpsimd states' broadcasts)
            for h in range(2):
                hsl = slice(h * 2048, (h + 1) * 2048)
                nc.sync.dma_start(bc_d[:, hsl], BCROWS[:, hsl])

            for seg, (c0, c1) in enumerate(SEGS):
                t0, t1 = c0 * CH, c1 * CH
                ssl = slice(t0, t1)
                slen = t1 - t0
                # broadcast B rows (and C rows) for pool states into SBUF
                brps = {}
                crps = {}
                for j in POOLJ:
                    brp = brcpool.tile([DI, slen], bf16, tag=f"brp{j}")
                    nc.sync.dma_start(
                        brp[:], bc_d[j:j + 1, ssl].to_broadcast((DI, slen)))
                    brps[j] = brp
                for j in POOLYJ:
                    crp = brcpool.tile([DI, slen], bf16, tag=f"crp{j}")
                    nc.sync.dma_start(
                        crp[:],
                        bc_d[8 + j:9 + j, ssl].to_broadcast((DI, slen)))
                    crps[j] = crp
                # scans for this segment
                for j in order:
                    dA = dapool.tile([DI, slen], f32, tag="dA")
                    nc.scalar.activation(dA[:], DT[:, ssl], Act.Exp,
                                         scale=a_sc[:, j:j + 1])
                    dbx = dbxpool.tile([DI, slen], bf16, tag="dbx")
                    if j in POOLJ:
                        nc.gpsimd.tensor_tensor(dbx[:], U[:, ssl],
                                                brps[j][:], op=Alu.mult)
                    else:
                        for ci in range(c0, c1):
                            bsl = slice(ci * CH, (ci + 1) * CH)
                            lsl = slice((ci - c0) * CH, (ci - c0 + 1) * CH)
                            br = pbc.tile([DI, CH], bf16, tag="bc")
                            nc.tensor.matmul(br[:],
                                             brep_lhsT[:, j * DI:(j + 1) * DI],
                                             XC[:, bsl], is_transpose=True)
                            nc.vector.tensor_tensor(dbx[:, lsl], U[:, bsl],
                                                    br[:], op=Alu.mult)
                    init = 0.0 if seg == 0 else Hs[j][:, t0 - 1:t0]
                    nc.vector.tensor_tensor_scan(Hs[j][:, ssl], dA[:], dbx[:],
                                                 init, op0=Alu.mult,
                                                 op1=Alu.add)
                # y for this segment
                for c in range(c0, c1):
                    y_chunk(c, crps, c0)
                nc.sync.dma_start(y_in_segs[seg][:], ysb[:, ssl])
                if sim:
                    nc.sync.dma_start(y_out_segs[seg][:], y_in_segs[seg][:])
                else:
                    nc.gpsimd.collective_compute(
                        "AllReduce", Alu.add, replica_groups=groups,
                        ins=[y_in_segs[seg].opt()],
                        outs=[y_out_segs[seg].opt()])
                nc.sync.dma_start(YSUM[:, ssl], y_out_segs[seg][:])
                # post of the PREVIOUS segment (its AR has landed by now)
                if seg >= 1:
                    emit_post(seg - 1)
            emit_post(len(SEGS) - 1)

    nc.compile()
    return nc


def _host_precompute(inp):
    import ml_dtypes
    f = lambda k: np.asarray(inp[k], np.float32)
    bf = lambda a: np.ascontiguousarray(a.astype(ml_dtypes.bfloat16))
    w1 = f("conv_w")[:, :, 0, 0]
    wh = f("dwh_w")[:, 0, :, 0]
    ww = f("dww_w")[:, 0, 0, :]
    taps = [
        w1 * (1.0 + wh[:, 1] + ww[:, 1])[None, :],   # center
        w1 * wh[:, 0][None, :],                       # up
        w1 * wh[:, 2][None, :],                       # down
        w1 * ww[:, 0][None, :],                       # left
        w1 * ww[:, 2][None, :],                       # right
    ]
    cw = np.concatenate([t.T for t in taps], axis=1)  # [64, 320]
    btot = f("conv_b") + w1 @ (f("dwh_b") + f("dww_b"))
    s_bn = f("bn_g") / np.sqrt(f("bn_v") + EPS)
    bn_bias = s_bn * (btot - f("bn_m")) + f("bn_b")
    ipw = f("in_proj_w")                               # [256, 64]
    ln_g = f("ln_g")
    ipx = ipw[:DI] * ln_g[None, :]                     # [128, 64]
    ipz = ipw[DI:] * ln_g[None, :]
    b_xm = ipw[:DI] @ f("ln_b")                        # [128]
    b_z = ipw[DI:] @ f("ln_b")
    cdw = f("convd_w")[:, 0, :]                        # [128, 4]
    # folded conv taps: Wk_lhsT[c, d] = ipx[d, c] * w_k[d]
    wk = np.concatenate([(ipx * cdw[:, k][:, None]).T for k in range(4)],
                        axis=1)                        # [64, 512]
    b_xc = cdw.sum(1) * b_xm + f("convd_b")
    # boundary corr for t in {0,1,2}: subtract (sum_{k<3-t} w_k) * b_xm
    corr = np.zeros((3, DI), np.float32)
    for t in range(3):
        corr[t] = -cdw[:, :3 - t].sum(1) * b_xm
    xpw = f("x_proj_w")                                # [36, 128]
    dtM = f("dt_proj_w") @ xpw[:DR]                    # [128, 128]
    a_full = -np.exp(np.asarray(inp["A_log"], np.float32))

    per_sigma = []
    for sg in range(2):
        s_lo = sg * NS
        cf32 = np.zeros((128, 16), np.float32)
        cf32[:C, 0] = s_bn
        cf32[:C, 1] = bn_bias
        cf32[:, 2] = b_z
        cf32[:, 3] = b_xc
        cf32[:, 4] = f("dt_proj_b")
        for j in range(NS):
            cf32[:, 5 + j] = a_full[:, s_lo + j]
        cf32[:, 13] = f("Dp")

        cbf = np.zeros((128, NCB), np.float32)
        cbf[:, 0:128] = np.eye(128, dtype=np.float32)
        cbf[:C, 128:448] = cw
        cbf[:C, 448:576] = ipz.T
        cbf[:C, 576:1088] = wk
        cbf[:, 1088:1216] = dtM.T
        for j in range(NS):
            s = s_lo + j
            cbf[:, 1216 + j * DI:1216 + (j + 1) * DI] = xpw[DR + s][:, None]
            cbf[:, 2240 + j * DI:2240 + (j + 1) * DI] = xpw[DR + DS + s][:, None]
        cbf[:, 3264:3328] = f("out_proj_w").T
        cbf[0:3, 3328:3456] = corr
        for j in range(NS):
            cbf[:, 3456 + j] = xpw[DR + s_lo + j]
            cbf[:, 3464 + j] = xpw[DR + DS + s_lo + j]
        per_sigma.append(dict(cf32=cf32, cbf=bf(cbf)))
    return {}, per_sigma


def _shift_images(xb):
    # 5 pre-shifted copies: ctr, up(reads h-1), dn(h+1), lf(w-1), rt(w+1)
    import ml_dtypes
    out = np.zeros((C, 5, H, W), np.float32)
    out[:, 0] = xb
    out[:, 1, 1:, :] = xb[:, :-1, :]
    out[:, 2, :-1, :] = xb[:, 1:, :]
    out[:, 3, :, 1:] = xb[:, :, :-1]
    out[:, 4, :, :-1] = xb[:, :, 1:]
    return np.ascontiguousarray(
        out.transpose(1, 0, 2, 3).reshape(5, C, L).transpose(1, 0, 2)
        .reshape(C, 5 * L).astype(ml_dtypes.bfloat16))


TRACE = False
LAST_EXEC_NS = None
LAST_TRACE_DIR = None


def kernel(**inputs):
    global LAST_EXEC_NS, LAST_TRACE_DIR
    from concourse.bass_utils import run_bass_kernel_spmd

    if "nc" not in _cached:
        _cached["nc"] = _build_program()
    nc = _cached["nc"]

    common, per_sigma = _host_precompute(inputs)
    x = np.asarray(inputs["x"], np.float32)
    in_maps = []
    for c in range(NCORES):
        b, sg = c // 2, c % 2
        m = dict(common)
        m.update(per_sigma[sg])
        m["ximgs"] = _shift_images(x[b])
        in_maps.append(m)

    kw = {}
    if TRACE:
        import tempfile
        LAST_TRACE_DIR = tempfile.mkdtemp(prefix="bass_trace_")
        kw = dict(trace=True, tmpdir=LAST_TRACE_DIR)
    r = run_bass_kernel_spmd(nc, in_maps, list(range(NCORES)), **kw)
    if r.exec_time_ns is not None:
        LAST_EXEC_NS = r.exec_time_ns
    res = r.results
    out = np.empty((B, C, H, W), np.float32)
    for b in range(B):
        out[b] = np.asarray(res[2 * b]["out_f"], np.float32).reshape(C, H, W)
    return out
